# revision 18
# baseline (speedup 1.0000x reference)
"""Trainium2 Bass kernel for the constrained leaky RNN.

Model (reference):
    x_proj = inputs @ W_in.T + b_in                    # [B, T, H]
    h_t    = (1-a)*h_{t-1} + a*tanh(x_proj_t + h_{t-1} @ W_rec.T + h_bias)
    out    = hs @ W_out.T + b_out                      # [B, T, O]
with B=64, T=2048, I=128, H=512, O=64, a=0.2.

Strategy (v5):
  - Data-parallel over batch: 8 cores x 8 batch rows, no collectives.
  - State transposed on-chip: H on partitions (4 tiles of 128), batch on
    the free dim.
  - Recurrence restructured so matmuls consume the tanh output directly:
        bank_{t+1} = xd-proj_{t+1} (+beta) + g_t @ (a W_rec).T   (PSUM)
        pre_{t+1}  = 0.8 * pre_t + bank_{t+1}                    (DVE STT)
        g_{t+1}    = tanh(pre_{t+1} / S)                         (ACT)
    with xd_t = x_t - 0.8 x_{t-1} differenced on the host so the decay of
    the input projection telescopes into the single pre recurrence, and
    h_t (only needed for the output projection) reconstructed off-path
    (Hs_t = 0.8 Hs_{t-1} + g_t).
  - TWO batch-staggered independent streams (rows 0-3 / 4-7 per core),
    each with its own PSUM bank pool and chain. One stream's serial chain
    (sem + STT + tanh) hides under the other stream's matmul burst, which
    is the only way past the ~1.2us/step single-stream chain floor.
  - Output projection batched per 32-step chunk, deferred into the next
    chunk's early steps.
"""

import os
import sys

sys.path.insert(0, "/opt/trn_rl_repo")

import numpy as np

B, T, I, H, O = 64, 2048, 128, 512, 64
NCORES = 8
BL = B // NCORES          # batch rows per core
NS = 2                    # independent staggered streams per core
BLs = BL // NS            # batch rows per stream
ALPHA = 0.2
DECAY = 1.0 - ALPHA
TC = 32                   # steps per chunk (xc DMA / hs buffer / outproj)
NCHUNK = T // TC

DT_REC = "fp16"           # weights/state dtype on chip
SCALE = 1.0               # scale folded into W_in/W_rec/beta; tanh scale=1/S

_BUILD_CACHE = {}


def _build(dt_flag: str, with_beta: bool):
    import concourse.tile as tile
    from concourse import bacc, mybir
    from contextlib import ExitStack

    f32 = mybir.dt.float32
    dt_rec = {"fp32": f32, "bf16": mybir.dt.bfloat16,
              "fp16": mybir.dt.float16}[dt_flag]
    Alu = mybir.AluOpType
    Act = mybir.ActivationFunctionType

    nc = bacc.Bacc("TRN2")
    xT = nc.dram_tensor("xT", [I, T * BL], dt_rec, kind="ExternalInput")
    wrecT = nc.dram_tensor("wrecT", [H, H], dt_rec, kind="ExternalInput")
    winT = nc.dram_tensor("winT", [I, H], dt_rec, kind="ExternalInput")
    beta = nc.dram_tensor("beta", [1, H], dt_rec, kind="ExternalInput")
    woutT = nc.dram_tensor("woutT", [H, O], dt_rec, kind="ExternalInput")
    bout = nc.dram_tensor("bout", [O, 1], f32, kind="ExternalInput")
    outT = nc.dram_tensor("outT", [O, T * BL], f32, kind="ExternalOutput")

    inv_s = 1.0 / SCALE

    with ExitStack() as ctx:
        tc = ctx.enter_context(tile.TileContext(nc))
        const = ctx.enter_context(tc.tile_pool(name="const", bufs=1))
        xpool = ctx.enter_context(tc.tile_pool(name="xpool", bufs=2))
        gpool = ctx.enter_context(tc.tile_pool(name="gpool", bufs=4))
        prepool = ctx.enter_context(tc.tile_pool(name="prepool", bufs=4))
        hspool = ctx.enter_context(tc.tile_pool(name="hspool", bufs=2))
        opool = ctx.enter_context(tc.tile_pool(name="opool", bufs=2))
        # PSUM banks: 2+2 per-stream slots + 2 outproj
        psS = [
            ctx.enter_context(tc.tile_pool(name="psL", bufs=2, space="PSUM")),
            ctx.enter_context(tc.tile_pool(name="psH", bufs=2, space="PSUM")),
        ]
        ps_o = ctx.enter_context(tc.tile_pool(name="ps_o", bufs=2, space="PSUM"))

        # ---- constants ----
        wrec_sb = const.tile([128, 4, H], dt_rec)       # [:, i, j*128+m]
        for i in range(4):
            nc.sync.dma_start(wrec_sb[:, i], wrecT[i * 128:(i + 1) * 128, :])
        win_sb = const.tile([I, H], dt_rec)
        nc.sync.dma_start(win_sb, winT[:, :])
        wout_sb = const.tile([128, 4, O], dt_rec)
        for j in range(4):
            nc.sync.dma_start(wout_sb[:, j], woutT[j * 128:(j + 1) * 128, :])
        bout_sb = const.tile([O, 1], f32)
        nc.sync.dma_start(bout_sb, bout[:, :])
        if with_beta:
            beta_sb = const.tile([1, H], dt_rec)
            nc.sync.dma_start(beta_sb, beta[:, :])
            bcoef_sb = const.tile([1, 2, BLs], dt_rec)
            nc.any.memset(bcoef_sb[:, 0], 1.0)
            nc.any.memset(bcoef_sb[:, 1], ALPHA)

        hs_init = const.tile([128, 4, BL], dt_rec)
        nc.any.memzero(hs_init[:])

        x_tiles = {}

        def load_chunk(c):
            if c >= NCHUNK:
                return
            xt = xpool.tile([I, TC * BL], dt_rec, tag="x")
            nc.sync.dma_start(xt, xT[:, c * TC * BL:(c + 1) * TC * BL])
            x_tiles[c] = xt

        load_chunk(0)
        load_chunk(1)

        def new_slot(s):
            full = psS[s].tile([128, 16, 4, BLs], f32, tag=f"slot{s}")
            return full[:, 0]

        def xd_beta_matmuls(t, s, slot):
            c, tl = divmod(t, TC)
            xc = x_tiles[c]
            base = tl * BL + s * BLs
            for j in range(4):
                nc.tensor.matmul(
                    slot[:, j],
                    win_sb[:, j * 128:(j + 1) * 128],
                    xc[:, base:base + BLs],
                    start=(j == 0), stop=False,
                    skip_group_check=True,
                )
            if with_beta:
                sel = 0 if t == 0 else 1
                for j in range(4):
                    nc.tensor.matmul(
                        slot[:, j],
                        beta_sb[:, j * 128:(j + 1) * 128],
                        bcoef_sb[:, sel],
                        start=False, stop=False,
                        skip_group_check=True,
                    )

        def outproj(hsc, oc):
            pso = ps_o.tile([O, TC * BL], f32, tag="pso")
            for j in range(4):
                nc.tensor.matmul(pso, wout_sb[:, j], hsc[:, j],
                                 start=(j == 0), stop=(j == 3))
            ob = opool.tile([O, TC * BL], f32, tag="ob")
            nc.scalar.activation(ob[:], pso, Act.Identity,
                                 bias=bout_sb[:, 0:1])
            nc.sync.dma_start(outT[:, oc * TC * BL:(oc + 1) * TC * BL], ob[:])

        # bootstrap: bank(0) = xp(0) for both streams
        slot = {}
        pre = {s: None for s in range(NS)}
        hs_prev = {}
        for s in range(NS):
            slot[s] = new_slot(s)
            xd_beta_matmuls(0, s, slot[s])
            hs_prev[s] = hs_init[:, :, s * BLs:(s + 1) * BLs]

        hs_chunk = None
        prev_hs_chunk = None

        for t in range(T):
            c, tl = divmod(t, TC)
            if tl == 0:
                prev_hs_chunk = hs_chunk
                hs_chunk = hspool.tile([128, 4, TC, BL], dt_rec, tag="hs")
                if t > 0:
                    load_chunk(c + 1)

            for s in range(NS):
                # pre_t = 0.8*pre_{t-1} + bank_t  (bank read once, on DVE)
                pre_new = prepool.tile([128, 4, BLs], f32, tag=f"pre{s}")
                if pre[s] is None:
                    nc.vector.tensor_scalar_mul(out=pre_new[:], in0=slot[s],
                                                scalar1=1.0)
                else:
                    nc.vector.scalar_tensor_tensor(
                        out=pre_new[:], in0=pre[s][:], scalar=DECAY,
                        in1=slot[s], op0=Alu.mult, op1=Alu.add,
                    )
                pre[s] = pre_new

                # g_t = tanh(pre_t / S)   (ACT, SBUF source - no PSUM clash)
                g = gpool.tile([128, 4, BLs], dt_rec, tag=f"g{s}")
                nc.scalar.activation(g[:], pre_new[:], Act.Tanh, scale=inv_s)

                # Hs_t = 0.8 * Hs_{t-1} + g_t   (off critical path)
                sl = slice(s * BLs, (s + 1) * BLs)
                nc.vector.scalar_tensor_tensor(
                    out=hs_chunk[:, :, tl, sl], in0=hs_prev[s], scalar=DECAY,
                    in1=g[:], op0=Alu.mult, op1=Alu.add,
                )
                hs_prev[s] = hs_chunk[:, :, tl, sl]

                if t < T - 1:
                    sn = new_slot(s)
                    xd_beta_matmuls(t + 1, s, sn)
                    for j in range(4):
                        for i in range(4):
                            nc.tensor.matmul(
                                sn[:, j],
                                wrec_sb[:, i, j * 128:(j + 1) * 128],
                                g[:, i],
                                start=False,
                                stop=(j == 3 and i == 3),
                                skip_group_check=True,
                            )
                    slot[s] = sn

            # deferred output projection for the previous chunk
            if tl == 2 and prev_hs_chunk is not None:
                outproj(prev_hs_chunk, c - 1)

        outproj(hs_chunk, NCHUNK - 1)

    nc.finalize()
    return nc


def _get_nc(dt_flag: str, with_beta: bool):
    key = (dt_flag, with_beta)
    if key not in _BUILD_CACHE:
        _BUILD_CACHE[key] = _build(dt_flag, with_beta)
    return _BUILD_CACHE[key]


def _prep_in_maps(inputs, dt_flag: str):
    import ml_dtypes

    x = np.asarray(inputs["inputs"], dtype=np.float32)
    W_in = np.asarray(inputs["W_in"], dtype=np.float32)
    b_in = np.asarray(inputs["b_in"], dtype=np.float32)
    W_rec = np.asarray(inputs["W_rec"], dtype=np.float32)
    h_bias = np.asarray(inputs["h_bias"], dtype=np.float32)
    W_out = np.asarray(inputs["W_out"], dtype=np.float32)
    b_out = np.asarray(inputs["b_out"], dtype=np.float32)

    dt = {"fp32": np.float32, "bf16": ml_dtypes.bfloat16,
          "fp16": np.float16}[dt_flag]

    # x differencing: xd_0 = x_0 ; xd_t = x_t - 0.8 x_{t-1}
    xd = x.copy()
    xd[:, 1:] -= DECAY * x[:, :-1]

    wrecT = np.ascontiguousarray((SCALE * ALPHA * W_rec.T).astype(dt))
    winT = np.ascontiguousarray((SCALE * W_in.T).astype(dt))
    beta_v = np.ascontiguousarray(
        (SCALE * (b_in + h_bias)).astype(dt).reshape(1, H))
    with_beta = bool(np.any(np.asarray(beta_v, dtype=np.float32) != 0))
    woutT = np.ascontiguousarray((ALPHA * W_out.T).astype(dt))
    bout = np.ascontiguousarray(b_out.reshape(O, 1))

    in_maps = []
    for c in range(NCORES):
        xc = xd[c * BL:(c + 1) * BL]                    # [BL, T, I]
        xTc = np.ascontiguousarray(
            xc.transpose(2, 1, 0).reshape(I, T * BL).astype(dt))
        in_maps.append({
            "xT": xTc, "wrecT": wrecT, "winT": winT,
            "beta": beta_v, "woutT": woutT, "bout": bout,
        })
    return in_maps, with_beta


def _run(inputs, trace=False, dt_flag=None, tmpdir=None):
    from concourse import bass_utils

    if dt_flag is None:
        dt_flag = DT_REC
    in_maps, with_beta = _prep_in_maps(inputs, dt_flag)
    nc = _get_nc(dt_flag, with_beta)
    res = bass_utils.run_bass_kernel_spmd(
        nc, in_maps, core_ids=list(range(NCORES)), trace=trace, tmpdir=tmpdir,
    )
    outs = []
    for c in range(NCORES):
        oT = res.results[c]["outT"]                     # [O, T*BL]
        outs.append(oT.reshape(O, T, BL).transpose(2, 1, 0))
    full = np.concatenate(outs, axis=0).astype(np.float32)
    return full, res


def kernel(**inputs) -> np.ndarray:
    out, _ = _run(inputs, trace=False)
    return out
